# revision 7
# baseline (speedup 1.0000x reference)
"""DiffOfGaussians Trainium2 kernel (v8: HW-queue streaming, transpose-free
layout, PE-only contraction, device-complete reduction).

Math:
  out[b,u] = sum_{h,w,c} inputs[b,h,w,c] * F[h,w,u] + bias[u]
  F[h,w,u] = g(a1,s1) - g(a2,s1+s2),  g(a,s) = a*exp(-((w-ux)^2+(h-uy)^2)/(2s))/(2*pi*s)

Separable: F[h,w,u] = sum_p sgn_p * coef_p[u] * gx_p[w,u] * gy_p[h,u].

Sharding: H split across 8 cores (16 rows each).  Host packs the shard as
bf16 [hp8, w128, (c16, b64, h2)] so that
  - w sits on SBUF partitions (no PE transpose anywhere),
  - the c-reduce is a 4-level unit-stride bf16 add tree (DVE 2x mode),
  - the matmul moving operand xr[w, (b,h)] is produced in place.
Input streams through the two HWDGE queues (sync + scalar), all 8 chunk
DMAs issued up front (whole shard lives in SBUF).  Per 8-h wave: 4 bf16
matmuls gx_p[w,u_half]^T @ xr -> psum[u, (b,h8)], DVE multiplies by the
gy table (broadcast over b) and a single tensor_reduce contracts h.
A final [128,(k2,b64)] fp32 tile is DMAed out; the host only sums the
8 per-core partials and adds the bias.
"""

import sys

for _p in ("/opt/trn_rl_repo",):
    if _p not in sys.path:
        sys.path.insert(0, _p)

import numpy as np

import concourse.bass as bass
import concourse.tile as tile
from concourse import bacc, mybir
from concourse.bass_utils import run_bass_kernel_spmd

F32 = mybir.dt.float32
BF16 = mybir.dt.bfloat16
OP = mybir.AluOpType
AF = mybir.ActivationFunctionType
AX = mybir.AxisListType

B, H, W, C, U = 64, 128, 128, 16, 256
NCORES = 8
HSH = H // NCORES      # 16 h rows per core
NCH = HSH // 2         # 8 chunks of 2 h rows
INV2PI = float(1.0 / (2.0 * np.pi))

_CACHE = {}


def _build_kernel():
    nc = bacc.Bacc(
        "TRN2",
        target_bir_lowering=False,
        debug=False,
        num_devices=NCORES,
    )

    # input chunks: x[hp, w, (c16, b64, h2)]; h = 16*core + 2*hp + h2
    x_d = nc.dram_tensor("x", [NCH, 128, 2048], BF16, kind="ExternalInput").ap()
    # broadcast row: cols 0:256 ux, 256:512 s1, 512:768 s2, 768:784 yc (h vals)
    aux_d = nc.dram_tensor("aux", [1, 784], F32, kind="ExternalInput").ap()
    # per-partition params (u_lo = partition, k = unit half):
    # cols (a1k0,a1k1,a2k0,a2k1,s1k0,s1k1,s2k0,s2k1,uyk0,uyk1)
    prm_d = nc.dram_tensor("prm", [128, 10], F32, kind="ExternalInput").ap()
    # out[u_lo, (k2, b64)] fp32 partial (host sums over cores, adds bias)
    out_d = nc.dram_tensor("out", [128, 128], F32, kind="ExternalOutput").ap()

    with tile.TileContext(nc) as tc:
        with (
            tc.tile_pool(name="singles", bufs=1) as sg,
            tc.tile_pool(name="xin", bufs=NCH) as xin_pool,
            tc.tile_pool(name="ta", bufs=2) as ta_pool,
            tc.tile_pool(name="tb", bufs=2) as tb_pool,
            tc.tile_pool(name="tcp", bufs=2) as tc_pool,
            tc.tile_pool(name="pz", bufs=8, space="PSUM") as pz_pool,
        ):
            # ---- input DMAs first so the streams start immediately ----
            aux_sb = sg.tile([128, 784], F32)
            aux_bc = bass.AP(
                tensor=aux_d.tensor, offset=aux_d.offset, ap=[[0, 128], [1, 784]]
            )
            nc.scalar.dma_start(out=aux_sb[:], in_=aux_bc)
            pp = sg.tile([128, 12], F32)
            nc.scalar.dma_start(out=pp[:, 0:10], in_=prm_d)

            xt = []
            for hp in range(NCH):
                t = xin_pool.tile([128, 2048], BF16, tag="x")
                eng = nc.sync if hp % 2 == 0 else nc.scalar
                eng.dma_start(out=t[:], in_=x_d[hp])
                xt.append(t)

            # ---- free-oriented (u along cols) param math ----
            ux_f = aux_sb[:, 0:256]
            s1_f = aux_sb[:, 256:512]
            s2_f = aux_sb[:, 512:768]
            yc_r = aux_sb[:, 768:784]

            niw_i = sg.tile([128, 1], mybir.dt.int32)
            nc.gpsimd.iota(niw_i[:], pattern=[[1, 1]], base=0, channel_multiplier=1)
            niw_f = sg.tile([128, 1], F32)
            nc.gpsimd.tensor_copy(niw_f[:], niw_i[:])
            niw = sg.tile([128, 1], F32)
            nc.gpsimd.tensor_scalar_mul(niw[:], niw_f[:], -1.0)

            sig_f = sg.tile([128, 256], F32)
            nc.vector.tensor_add(sig_f[:], s1_f, s2_f)
            rc1_f = sg.tile([128, 256], F32)
            nc.vector.reciprocal(rc1_f[:], s1_f)
            rc2_f = sg.tile([128, 256], F32)
            nc.vector.reciprocal(rc2_f[:], sig_f[:])
            nis1_f = sg.tile([128, 256], F32)
            nc.vector.tensor_scalar_mul(nis1_f[:], rc1_f[:], -0.5)
            nis2_f = sg.tile([128, 256], F32)
            nc.vector.tensor_scalar_mul(nis2_f[:], rc2_f[:], -0.5)

            # d2[w,u] = (ux[u] - w)^2 ; shared by both paths
            d2 = sg.tile([128, 256], F32)
            nc.scalar.activation(d2[:], ux_f, AF.Square, bias=niw[:, 0:1])

            e1 = sg.tile([128, 256], F32)
            nc.vector.tensor_tensor(e1[:], d2[:], nis1_f[:], op=OP.mult)
            e2 = sg.tile([128, 256], F32)
            nc.vector.tensor_tensor(e2[:], d2[:], nis2_f[:], op=OP.mult)

            gx = []
            for p, e in enumerate((e1, e2)):
                g = sg.tile([128, 256], BF16, tag=f"gx{p}")
                nc.scalar.activation(g[:], e[:], AF.Exp, bias=0.0)
                gx.append(g)

            # ---- per-partition (u_lo) param math for gy ----
            # pp cols: 0:4 = a(p,k), 4:6 s1, 6:8 s2, 8:10 uy, 10:12 scratch sig
            nc.gpsimd.tensor_add(pp[:, 10:12], pp[:, 4:6], pp[:, 6:8])
            rct = sg.tile([128, 4], F32)  # 1/sigma, cols (p,k)
            nc.vector.reciprocal(rct[:, 0:2], pp[:, 4:6])
            nc.vector.reciprocal(rct[:, 2:4], pp[:, 10:12])
            nis4 = sg.tile([128, 4], F32)
            nc.gpsimd.tensor_scalar_mul(nis4[:], rct[:], -0.5)
            coef4 = sg.tile([128, 4], F32)
            nc.gpsimd.tensor_tensor(coef4[:], pp[:, 0:4], rct[:], op=OP.mult)
            nc.gpsimd.tensor_scalar_mul(coef4[:, 0:2], coef4[:, 0:2], INV2PI)
            nc.gpsimd.tensor_scalar_mul(coef4[:, 2:4], coef4[:, 2:4], -INV2PI)
            nuy2 = sg.tile([128, 2], F32)
            nc.gpsimd.tensor_scalar_mul(nuy2[:], pp[:, 8:10], -1.0)

            # gy_sb[u_lo, (p2, k2, h16)] bf16, with sign+coef folded in
            gy_sb = sg.tile([128, 64], BF16)
            with nc.allow_low_precision("bf16 tables; 2e-2 rel-err budget"):
                for k in range(2):
                    dy2 = sg.tile([128, 16], F32, tag=f"dy2_{k}")
                    nc.scalar.activation(
                        dy2[:], yc_r, AF.Square, bias=nuy2[:, k : k + 1]
                    )
                    for p in range(2):
                        eg = sg.tile([128, 16], F32, tag=f"eg{p}{k}")
                        nc.scalar.activation(
                            eg[:], dy2[:], AF.Exp, bias=0.0,
                            scale=nis4[:, 2 * p + k : 2 * p + k + 1],
                        )
                        nc.gpsimd.tensor_scalar_mul(
                            gy_sb[:, (2 * p + k) * 16 : (2 * p + k) * 16 + 16],
                            eg[:], coef4[:, 2 * p + k : 2 * p + k + 1],
                        )

            # ---- c-reduce tree: x[w,(c16,b64,h2)] -> xr[w,(W2,b64,h8)] ----
            xr = sg.tile([128, 1024], BF16)
            with nc.allow_low_precision("bf16 c-reduce; 2e-2 rel-err budget"):
                for hp in range(NCH):
                    t = xt[hp]
                    a = ta_pool.tile([128, 1024], BF16, tag="a")
                    nc.vector.tensor_add(a[:], t[:, 0:1024], t[:, 1024:2048])
                    b_ = tb_pool.tile([128, 512], BF16, tag="b")
                    nc.vector.tensor_add(b_[:], a[:, 0:512], a[:, 512:1024])
                    c_ = tc_pool.tile([128, 256], BF16, tag="c")
                    nc.gpsimd.tensor_add(c_[:], b_[:, 0:256], b_[:, 256:512])
                    # write [w, b64, h2] into xr cols W*512 + b*8 + (hp%4)*2
                    wv = hp // 4
                    dst = bass.AP(
                        tensor=xr.tensor,
                        offset=xr.offset + wv * 512 + (hp % 4) * 2,
                        ap=[xr.ap[0], [8, 64], [1, 2]],
                    )
                    s0 = c_[:, 0:128].rearrange("q (b h) -> q b h", h=2)
                    s1 = c_[:, 128:256].rearrange("q (b h) -> q b h", h=2)
                    nc.gpsimd.tensor_add(dst, s0, s1)

            # ---- waves: matmul + gy mult + h-reduce ----
            vws = []
            with nc.allow_low_precision("bf16 partials; 2e-2 rel-err budget"):
                for wv in range(2):
                    zt = sg.tile([128, 2048], BF16, tag=f"z{wv}")
                    for k in range(2):
                        for p in range(2):
                            z = pz_pool.tile([128, 512], F32, tag="z")
                            nc.tensor.matmul(
                                z[:],
                                gx[p][:, k * 128 : (k + 1) * 128],
                                xr[:, wv * 512 : (wv + 1) * 512],
                                start=True,
                                stop=True,
                            )
                            sl = gy_sb[:, (2 * p + k) * 16 + wv * 8 :]
                            gb = bass.AP(
                                tensor=sl.tensor, offset=sl.offset,
                                ap=[sl.ap[0], [0, 64], [1, 8]],
                            )
                            nc.vector.tensor_tensor(
                                zt[:, (2 * k + p) * 512 : (2 * k + p) * 512 + 512]
                                .rearrange("q (b h) -> q b h", h=8),
                                z[:].rearrange("q (b h) -> q b h", h=8),
                                gb, op=OP.mult,
                            )
                    vw = sg.tile([128, 256], F32, tag=f"vw{wv}")
                    nc.vector.tensor_reduce(
                        vw[:].rearrange("q (g x) -> q g x", x=1),
                        zt[:].rearrange("q (g h) -> q g h", h=8),
                        axis=AX.X, op=OP.add,
                    )
                    vws.append(vw)

            # sum the two paths within each wave, then the waves
            acc = []
            for wv in range(2):
                a_ = sg.tile([128, 128], F32, tag=f"acc{wv}")
                v = vws[wv].rearrange("q (k p b) -> q k p b", k=2, p=2)
                nc.vector.tensor_add(
                    a_[:].rearrange("q (k b) -> q k b", k=2),
                    v[:, :, 0, :], v[:, :, 1, :],
                )
                acc.append(a_)
            vout = sg.tile([128, 128], F32)
            nc.vector.tensor_add(vout[:], acc[0][:], acc[1][:])
            nc.scalar.dma_start(out=out_d, in_=vout[:])

    nc.compile()
    return nc


def _get_nc():
    if "nc" not in _CACHE:
        _CACHE["nc"] = _build_kernel()
    return _CACHE["nc"]


def pack_x(x: np.ndarray) -> np.ndarray:
    """[B,H,W,C] fp32 -> bf16 [core, hp, w, (c16, b64, h2)]."""
    import ml_dtypes

    xb = x.astype(ml_dtypes.bfloat16)
    # [B,H,W,C] -> [W,C,H,B]
    xb = xb.transpose(2, 3, 1, 0)
    # -> [W, C, core, hp, h2, B]
    xb = xb.reshape(W, C, NCORES, NCH, 2, B)
    # -> [core, hp, W, C, B, h2]
    xb = xb.transpose(2, 3, 0, 1, 5, 4)
    return np.ascontiguousarray(xb.reshape(NCORES, NCH, W, C * B * 2))


def pack_host(inputs: dict):
    x = np.asarray(inputs["inputs"], dtype=np.float32)
    xp = pack_x(x)
    p = {n: np.asarray(inputs[n], dtype=np.float32).reshape(U)
         for n in ("a1", "a2", "s1", "s2", "ux", "uy")}
    prm = np.zeros((128, 10), dtype=np.float32)
    for i, n in enumerate(("a1", "a2", "s1", "s2", "uy")):
        prm[:, 2 * i] = p[n][:128]
        prm[:, 2 * i + 1] = p[n][128:]
    aux = np.zeros((NCORES, 1, 784), dtype=np.float32)
    for c in range(NCORES):
        aux[c, 0, 0:256] = p["ux"]
        aux[c, 0, 256:512] = p["s1"]
        aux[c, 0, 512:768] = p["s2"]
        aux[c, 0, 768:784] = np.arange(c * HSH, (c + 1) * HSH, dtype=np.float32)
    return xp, aux, prm


def run(inputs: dict, trace: bool = False):
    """Run on 8 cores; returns (full_output, BassKernelResults)."""
    nc = _get_nc()
    xp, aux, prm = pack_host(inputs)
    in_maps = [
        {"x": xp[i], "aux": aux[i], "prm": prm} for i in range(NCORES)
    ]
    res = run_bass_kernel_spmd(
        nc, in_maps, core_ids=list(range(NCORES)), trace=trace
    )
    # per core: out[u_lo, (k2, b64)] fp32
    total = np.zeros((128, 2, 64), dtype=np.float64)
    for r in res.results:
        total += r["out"].astype(np.float64).reshape(128, 2, 64)
    # out[b, k*128 + u_lo]
    out = total.transpose(2, 1, 0).reshape(B, U)
    out = out + np.asarray(inputs["bias"], dtype=np.float64).reshape(1, U)
    return out.astype(np.float32), res


def kernel(**inputs) -> np.ndarray:
    out, _ = run(inputs, trace=False)
    return out


# revision 14
# speedup vs baseline: 1.0719x; 1.0719x over previous
"""DiffOfGaussians Trainium2 kernel (v9: paired-chunk pipeline, per-pair
sub-waves, engine-balanced reduction).

Math:
  out[b,u] = sum_{h,w,c} inputs[b,h,w,c] * F[h,w,u] + bias[u]
  F[h,w,u] = sum_p sgn_p * coef_p[u] * gx_p[w,u] * gy_p[h,u]   (separable)

Sharding: H split across 8 cores (16 rows each).  Host packs the shard as
bf16 [hp8, w128, (c16, b64, h2)]: w on SBUF partitions (no transposes of
the data anywhere), c outermost so the c-reduce is a unit-stride bf16 add
tree in DVE 2x mode.  The two HWDGE queues stream all 8 chunks up front
into one big SBUF tile.  Chunks are processed in pairs (h4 granularity):
  L1+L2 (DVE) -> L3+L4 (GpSimd) -> 4 matmuls f=256 (PE, one per p,k)
  -> scalar copies PSUM->SBUF bf16 -> one 4D DVE mult by the gy table
  -> h4 tree + path-subtract (DVE/GpSimd) -> fp32 accumulator.
gx tables are built per-partition-u (scalar activations) and flipped with
PE transposes; gy stays per-partition-u.  One [128,(k2,b64)] fp32 DMA out;
the host sums 8 per-core partials and adds the bias.
"""

import sys

for _p in ("/opt/trn_rl_repo",):
    if _p not in sys.path:
        sys.path.insert(0, _p)

import numpy as np

import concourse.bass as bass
import concourse.tile as tile
from concourse import bacc, masks, mybir
from concourse.bass_utils import run_bass_kernel_spmd

F32 = mybir.dt.float32
BF16 = mybir.dt.bfloat16
I32 = mybir.dt.int32
OP = mybir.AluOpType
AF = mybir.ActivationFunctionType

B, H, W, C, U = 64, 128, 128, 16, 256
NCORES = 8
HSH = H // NCORES      # 16 h rows per core
NCH = HSH // 2         # 8 chunks of 2 h rows
NPAIR = NCH // 2       # 4 pairs of chunks (h4 each)
INV2PI = float(1.0 / (2.0 * np.pi))

_CACHE = {}


def _ap(t, off, dims):
    """AP over tile t at element offset off with free dims [(stride, n), ...]."""
    base = t[:]
    return bass.AP(
        tensor=base.tensor, offset=base.offset + off,
        ap=[base.ap[0]] + [list(d) for d in dims],
    )


def _build_kernel():
    nc = bacc.Bacc(
        "TRN2",
        target_bir_lowering=False,
        debug=False,
        num_devices=NCORES,
    )

    # input chunks: x[hp, w, (c16, b64, h2)]; h = 16*core + 2*hp + h2
    x_d = nc.dram_tensor("x", [NCH, 128, 2048], BF16, kind="ExternalInput").ap()
    # yc broadcast row (per-core h values)
    aux_d = nc.dram_tensor("aux", [1, 16], F32, kind="ExternalInput").ap()
    # per-partition params: (a1k0,a1k1,a2k0,a2k1,s1k0,s1k1,s2k0,s2k1,uyk0,uyk1,uxk0,uxk1)
    prm_d = nc.dram_tensor("prm", [128, 12], F32, kind="ExternalInput").ap()
    # out[u_lo, (k2, b64)] fp32 partial
    out_d = nc.dram_tensor("out", [128, 128], F32, kind="ExternalOutput").ap()

    with tile.TileContext(nc) as tc:
        with (
            tc.tile_pool(name="singles", bufs=1) as sg,
            tc.tile_pool(name="ta", bufs=2) as ta_pool,
            tc.tile_pool(name="tb", bufs=2) as tb_pool,
            tc.tile_pool(name="tcp", bufs=2) as tc_pool,
            tc.tile_pool(name="zq", bufs=2) as zq_pool,
            tc.tile_pool(name="ep", bufs=2) as ep_pool,
            tc.tile_pool(name="ptr", bufs=2, space="PSUM") as tr_psum,
            tc.tile_pool(name="pz", bufs=6, space="PSUM") as pz_pool,
        ):
            # ---- input DMAs first: both HW queues stream the whole shard ----
            X = sg.tile([128, NCH * 2048], BF16)
            for hp in range(NCH):
                eng = nc.sync if hp % 2 == 0 else nc.scalar
                eng.dma_start(out=X[:, hp * 2048 : (hp + 1) * 2048], in_=x_d[hp])

            # small inputs via gpsimd SWDGE (keeps HW queues pure)
            pp = sg.tile([128, 14], F32)
            nc.gpsimd.dma_start(out=pp[:, 0:12], in_=prm_d)
            yc_r = sg.tile([128, 16], F32)
            aux_bc = bass.AP(
                tensor=aux_d.tensor, offset=aux_d.offset, ap=[[0, 128], [1, 16]]
            )
            nc.gpsimd.dma_start(out=yc_r[:], in_=aux_bc)

            # ---- constants ----
            identity = sg.tile([128, 128], BF16)
            masks.make_identity(nc, identity[:])
            iota_i = sg.tile([128, 128], I32)
            nc.gpsimd.iota(iota_i[:], pattern=[[1, 128]], base=0, channel_multiplier=0)
            iota_f = sg.tile([128, 128], F32)
            nc.gpsimd.tensor_copy(iota_f[:], iota_i[:])

            # ---- per-partition (u_lo) param math ----
            # pp cols 12:14 = sig = s1+s2
            nc.gpsimd.tensor_add(pp[:, 12:14], pp[:, 4:6], pp[:, 6:8])
            rct = sg.tile([128, 4], F32)  # 1/sigma, cols (p,k)
            nc.vector.reciprocal(rct[:, 0:2], pp[:, 4:6])
            nc.vector.reciprocal(rct[:, 2:4], pp[:, 12:14])
            nis4 = sg.tile([128, 4], F32)
            nc.gpsimd.tensor_scalar_mul(nis4[:], rct[:], -0.5)
            coef4 = sg.tile([128, 4], F32)
            nc.gpsimd.tensor_tensor(coef4[:], pp[:, 0:4], rct[:], op=OP.mult)
            nc.gpsimd.tensor_scalar_mul(coef4[:, 0:2], coef4[:, 0:2], INV2PI)
            nc.gpsimd.tensor_scalar_mul(coef4[:, 2:4], coef4[:, 2:4], -INV2PI)
            nuy2 = sg.tile([128, 2], F32)
            nc.gpsimd.tensor_scalar_mul(nuy2[:], pp[:, 8:10], -1.0)
            nux2 = sg.tile([128, 2], F32)
            nc.gpsimd.tensor_scalar_mul(nux2[:], pp[:, 10:12], -1.0)

            # ---- gx tables: build [u_lo, w] then PE-transpose to [w, u] ----
            gx = []
            for p in range(2):
                g = sg.tile([128, 256], BF16, tag=f"gx{p}")
                gx.append(g)
            with nc.allow_low_precision("bf16 tables; 2e-2 rel-err budget"):
                for k in range(2):
                    d2 = sg.tile([128, 128], F32, tag=f"d2_{k}")
                    nc.scalar.activation(
                        d2[:], iota_f[:], AF.Square, bias=nux2[:, k : k + 1]
                    )
                    for p in range(2):
                        e = ta_pool.tile([128, 128], BF16, tag="gxe")
                        nc.scalar.activation(
                            e[:], d2[:], AF.Exp, bias=0.0,
                            scale=nis4[:, 2 * p + k : 2 * p + k + 1],
                        )
                        ps = tr_psum.tile([128, 128], BF16, tag="trp")
                        nc.tensor.transpose(ps[:], e[:], identity[:])
                        nc.scalar.activation(
                            gx[p][:, k * 128 : (k + 1) * 128], ps[:], AF.Copy
                        )

                # ---- gy table [u_lo, (p2, k2, h16)] bf16, sign+coef folded ----
                gy_sb = sg.tile([128, 64], BF16)
                for k in range(2):
                    dy2 = sg.tile([128, 16], F32, tag=f"dy2_{k}")
                    nc.scalar.activation(
                        dy2[:], yc_r[:], AF.Square, bias=nuy2[:, k : k + 1]
                    )
                    for p in range(2):
                        eg = sg.tile([128, 16], F32, tag=f"eg{p}{k}")
                        nc.scalar.activation(
                            eg[:], dy2[:], AF.Exp, bias=0.0,
                            scale=nis4[:, 2 * p + k : 2 * p + k + 1],
                        )
                        nc.gpsimd.tensor_scalar_mul(
                            gy_sb[:, k * 32 + p * 16 : k * 32 + p * 16 + 16],
                            eg[:], coef4[:, 2 * p + k : 2 * p + k + 1],
                        )

                # ---- fp32 accumulator & xr [w, (wave2, b64, h8)] ----
                acc = sg.tile([128, 128], F32)
                nc.vector.memset(acc[:], 0.0)
                xr = sg.tile([128, 1024], BF16)

                # ---- per-pair pipeline ----
                for q in range(NPAIR):
                    wv, hq = q // 2, q % 2
                    # L1: chunks {2q, 2q+1} c16->c8, into (c8, chunk2, b64, h2)
                    a = ta_pool.tile([128, 2048], BF16, tag="a")
                    # src dims (chunk2, c8, bh128); X cols (hp, c16, b64, h2)
                    s0 = _ap(X, q * 4096, [(2048, 2), (128, 8), (1, 128)])
                    s1 = _ap(X, q * 4096 + 1024, [(2048, 2), (128, 8), (1, 128)])
                    d = _ap(a, 0, [(128, 2), (256, 8), (1, 128)])
                    nc.vector.tensor_tensor(d, s0, s1, op=OP.add)
                    # L2: contiguous halves (c8 -> c4)
                    b_ = tb_pool.tile([128, 1024], BF16, tag="b")
                    nc.vector.tensor_add(b_[:], a[:, 0:1024], a[:, 1024:2048])
                    # L3 on gpsimd (c4 -> c2)
                    c_ = tc_pool.tile([128, 512], BF16, tag="c")
                    nc.gpsimd.tensor_add(c_[:], b_[:, 0:512], b_[:, 512:1024])
                    # L4 on gpsimd (c2 -> 1) -> xr[:, wv*512 + b*8 + hq*4 + ch*2 + h2]
                    # Ct cols (c2, chunk2, b64, h2)
                    dstx = _ap(xr, wv * 512 + hq * 4, [(2, 2), (8, 64), (1, 2)])
                    sc0 = _ap(c_, 0, [(128, 2), (2, 64), (1, 2)])
                    sc1 = _ap(c_, 256, [(128, 2), (2, 64), (1, 2)])
                    nc.gpsimd.tensor_tensor(dstx, sc0, sc1, op=OP.add)

                    # 4 matmuls f=256: psum zk[u, (p2, b64, h4)] per k
                    rhs = _ap(xr, wv * 512 + hq * 4, [(8, 64), (1, 4)])
                    zql = []
                    for k in range(2):
                        z = pz_pool.tile([128, 512], F32, tag="z")
                        for p in range(2):
                            nc.tensor.matmul(
                                z[:, p * 256 : (p + 1) * 256],
                                gx[p][:, k * 128 : (k + 1) * 128],
                                rhs,
                                start=True,
                                stop=True,
                            )
                        zql.append(z)
                    # scalar: PSUM -> SBUF bf16, zq cols (k2, p2, b64, h4)
                    zq = zq_pool.tile([128, 1024], BF16, tag="zq")
                    for k in range(2):
                        nc.scalar.activation(
                            zq[:, k * 512 : (k + 1) * 512], zql[k][:], AF.Copy
                        )
                    # mult by gy (broadcast over b); (k,p) merged: zq stride
                    # 256, gy_sb cols (k2,p2,h16) stride 16
                    e0 = ep_pool.tile([128, 1024], BF16, tag="e0")
                    zin = _ap(zq, 0, [(256, 4), (4, 64), (1, 4)])
                    gyb = _ap(gy_sb, 4 * q, [(16, 4), (0, 64), (1, 4)])
                    eo = _ap(e0, 0, [(256, 4), (4, 64), (1, 4)])
                    nc.vector.tensor_tensor(eo, zin, gyb, op=OP.mult)
                    # h4 tree: (kp4, b64, h4) -> h2 -> h1
                    t1 = ep_pool.tile([128, 512], BF16, tag="t1")
                    i0 = _ap(e0, 0, [(256, 4), (4, 64), (1, 2)])
                    i1 = _ap(e0, 2, [(256, 4), (4, 64), (1, 2)])
                    to = _ap(t1, 0, [(128, 4), (2, 64), (1, 2)])
                    eng1 = nc.vector if q % 2 == 0 else nc.gpsimd
                    eng1.tensor_tensor(to, i0, i1, op=OP.add)
                    t2 = ep_pool.tile([128, 256], BF16, tag="t2")
                    j0 = _ap(t1, 0, [(128, 4), (2, 64)])
                    j1 = _ap(t1, 1, [(128, 4), (2, 64)])
                    ko = _ap(t2, 0, [(64, 4), (1, 64)])
                    eng1.tensor_tensor(ko, j0, j1, op=OP.add)
                    # sum paths, accumulate into acc[u, (k2, b64)] fp32
                    ctb = ep_pool.tile([128, 128], F32, tag="ctb")
                    p0 = _ap(t2, 0, [(128, 2), (1, 64)])
                    p1 = _ap(t2, 64, [(128, 2), (1, 64)])
                    co = _ap(ctb, 0, [(64, 2), (1, 64)])
                    eng2 = nc.gpsimd if q % 2 == 0 else nc.vector
                    eng2.tensor_tensor(co, p0, p1, op=OP.add)
                    nc.vector.tensor_add(acc[:], acc[:], ctb[:])

            nc.sync.dma_start(out=out_d, in_=acc[:])

    nc.compile()
    return nc


def _patch_xr(src: str) -> str:
    return src


def _get_nc():
    if "nc" not in _CACHE:
        _CACHE["nc"] = _build_kernel()
    return _CACHE["nc"]


def pack_x(x: np.ndarray) -> np.ndarray:
    """[B,H,W,C] fp32 -> bf16 [core, hp, w, (c16, b64, h2)]."""
    import ml_dtypes

    xb = x.astype(ml_dtypes.bfloat16)
    xb = xb.transpose(2, 3, 1, 0)                 # [W, C, H, B]
    xb = xb.reshape(W, C, NCORES, NCH, 2, B)
    xb = xb.transpose(2, 3, 0, 1, 5, 4)            # [core, hp, W, C, B, h2]
    return np.ascontiguousarray(xb.reshape(NCORES, NCH, W, C * B * 2))


def pack_host(inputs: dict):
    x = np.asarray(inputs["inputs"], dtype=np.float32)
    xp = pack_x(x)
    p = {n: np.asarray(inputs[n], dtype=np.float32).reshape(U)
         for n in ("a1", "a2", "s1", "s2", "ux", "uy")}
    prm = np.zeros((128, 12), dtype=np.float32)
    for i, n in enumerate(("a1", "a2", "s1", "s2", "uy", "ux")):
        prm[:, 2 * i] = p[n][:128]
        prm[:, 2 * i + 1] = p[n][128:]
    aux = np.zeros((NCORES, 1, 16), dtype=np.float32)
    for c in range(NCORES):
        aux[c, 0, :] = np.arange(c * HSH, (c + 1) * HSH, dtype=np.float32)
    return xp, aux, prm


def run(inputs: dict, trace: bool = False):
    """Run on 8 cores; returns (full_output, BassKernelResults)."""
    nc = _get_nc()
    xp, aux, prm = pack_host(inputs)
    in_maps = [
        {"x": xp[i], "aux": aux[i], "prm": prm} for i in range(NCORES)
    ]
    res = run_bass_kernel_spmd(
        nc, in_maps, core_ids=list(range(NCORES)), trace=trace
    )
    total = np.zeros((128, 2, 64), dtype=np.float64)
    for r in res.results:
        total += r["out"].astype(np.float64).reshape(128, 2, 64)
    out = total.transpose(2, 1, 0).reshape(B, U)
    out = out + np.asarray(inputs["bias"], dtype=np.float64).reshape(1, U)
    return out.astype(np.float32), res


def kernel(**inputs) -> np.ndarray:
    out, _ = run(inputs, trace=False)
    return out
